# revision 7
# baseline (speedup 1.0000x reference)
# Distributed causal self-attention for 8 Trainium2 NeuronCores.
#
# Problem: B=2, T=2048, C=768, H=12 heads, D=64. y = proj(attn(qkv(x))).
#
# Sharding: 2 (batch) x 4 (head-groups of 3 heads). Core c handles batch
# c//4 and heads (c%4)*3 .. +3. Each core computes its slice of the QKV
# projection, full attention for its 3 heads, and a partial output
# projection y_part = O_heads @ Wp_slice.T. Host sums the 4 partials per
# batch and adds b_proj.
#
# Device-side structure (v2 — single flat pipeline, PE-bound):
#   - xt loaded in 2 large multi-queue DMAs (3D APs), not 24 small ones.
#   - QKV emits per-head [q|k] packed M=128 outputs in [128,512] chunks;
#     q,k are then row-duplicated (rows 64:128 = rows 0:64) via SBUF->SBUF
#     DMA so attention S-matmuls (K=64) can be issued in PAIRS to PE row
#     groups (0,0) and (64,0) via tile_position -> both run concurrently.
#   - Each S-pair writes one [128,2048] PSUM tile (A at 0:1024, B at
#     1024:2048); ONE batched exp per pair on ScalarE (halves the 352-cycle
#     per-instruction overhead), split in two when the dead gap between the
#     two valid regions exceeds the overhead.
#   - QKV for later heads, v-projection units, norm chains and proj tiles
#     are emitted as PE filler INSIDE the attention stream, so the
#     ScalarE-bound attention phase overlaps the PE-bound projections.
#   - Pass-major block order (all heads p0, then all heads p1) so proj
#     tiles for cols 0:1024 unlock after the third block.
#   PSUM: sp [128,2048] (4 banks) + ot [128,1024] (2) + fill [128,512]x2
#   (2, shared by qkv/v/proj/norm-bs units) = exactly 8 banks.

import numpy as np

B, T, C, H, D = 2, 2048, 768, 12, 64
HPG = 3                      # heads per group
G = 4                        # head groups
CPG = HPG * D                # 192 channels per group
KT = C // 128                # 6 contraction tiles for projections
NT = T // 128                # 16 seq tiles
PW = 1024                    # tq pass width
SCALE = float(1.0 / np.sqrt(2.0))   # 1/sqrt(B) (faithful to reference)

_CACHE = {}


def _build_module():
    import concourse.bass as bass
    import concourse.tile as tile
    import concourse.mybir as mybir
    from concourse.bacc import Bacc
    from contextlib import ExitStack

    f32 = mybir.dt.float32
    bf16 = mybir.dt.bfloat16
    AF = mybir.ActivationFunctionType

    nc = Bacc()

    xt_d = nc.dram_tensor("xt", [C, T], bf16, kind="ExternalInput")
    wqkt_d = nc.dram_tensor("wqkt", [C, HPG * 128], bf16, kind="ExternalInput")
    wvt_d = nc.dram_tensor("wvt", [C, CPG], bf16, kind="ExternalInput")
    bqk_d = nc.dram_tensor("bqk", [128, HPG], f32, kind="ExternalInput")
    bv_d = nc.dram_tensor("bv", [128, CPG], f32, kind="ExternalInput")
    wpt_d = nc.dram_tensor("wpt", [CPG, C], bf16, kind="ExternalInput")
    mask_d = nc.dram_tensor("mask", [128, 128], bf16, kind="ExternalInput")
    y_d = nc.dram_tensor("y", [T, C], bf16, kind="ExternalOutput")

    with tile.TileContext(nc) as tc, ExitStack() as ctx:
        sb = ctx.enter_context(tc.tile_pool(name="sb", bufs=1))
        ps = ctx.enter_context(tc.tile_pool(name="ps", bufs=1, space="PSUM"))

        def fill_tile(name):
            return ps.tile([128, 512], f32, tag="fill", bufs=2, name=name)

        # ---- weights / constants into SBUF ----
        # wqkt + first half of xt gate the first matmul: one big DMA each
        # on the two HWDGE queues (sync, scalar).
        wqkt_sb = sb.tile([128, KT * HPG * 128], bf16, tag="wqk", name="wqkt")
        M3 = HPG * 128
        for k in range(KT):
            nc.sync.dma_start(wqkt_sb[:, k * M3:(k + 1) * M3],
                              wqkt_d[k * 128:(k + 1) * 128, :])
        xt_sb = sb.tile([128, KT * T], bf16, tag="xt", name="xt")
        xt3 = xt_sb[:, :].rearrange("p (k t) -> p k t", k=KT)
        for k in range(KT):
            nc.scalar.dma_start(xt3[:, k, 0:1024],
                                xt_d[k * 128:(k + 1) * 128, 0:1024])
        wvt_sb = sb.tile([128, KT * CPG], bf16, tag="wv", name="wvt")
        for k in range(KT):
            nc.sync.dma_start(wvt_sb[:, k * CPG:(k + 1) * CPG],
                              wvt_d[k * 128:(k + 1) * 128, :])
        for k in range(KT):
            nc.sync.dma_start(xt3[:, k, 1024:2048],
                              xt_d[k * 128:(k + 1) * 128, 1024:2048])
        bqk_sb = sb.tile([128, HPG], f32, tag="bqk", name="bqk")
        nc.scalar.dma_start(bqk_sb[:, :], bqk_d[:, :])
        bv_sb = sb.tile([128, CPG], f32, tag="bv", name="bv")
        nc.scalar.dma_start(bv_sb[:, :], bv_d[:, :])
        mask_sb = sb.tile([128, 128], bf16, tag="mask", name="mask")
        nc.gpsimd.dma_start(mask_sb[:, :], mask_d[:, :])
        wpt0_sb = sb.tile([128, C], bf16, tag="wpt0", name="wpt0")
        nc.gpsimd.dma_start(wpt0_sb[:, :], wpt_d[0:128, :])
        # K-pad second proj K-tile to 128 rows of zeros (full PE array).
        wpt1_sb = sb.tile([128, C], bf16, tag="wpt1", name="wpt1")
        nc.gpsimd.memset(wpt1_sb[64:128, :], 0.0)
        nc.gpsimd.dma_start(wpt1_sb[0:64, :], wpt_d[128:CPG, :])
        ones_sb = sb.tile([1, 128], bf16, tag="ones", name="ones")
        nc.vector.memset(ones_sb[:, :], 1.0)
        # Warm the ScalarE exp spline table before attention needs it.
        expwarm = sb.tile([1, 128], f32, tag="expwarm", name="expwarm")
        nc.scalar.activation(expwarm[:, :], ones_sb[:, :], AF.Exp)

        # v storage: one big tile, [v(64) | ones(1) | zeros(63)] per
        # (token-tile, head); pads pre-set ONCE with two strided memsets.
        vall = sb.tile([128, NT * HPG * 128], bf16, tag="vall", name="vall")
        v4 = vall[:, :].rearrange("p (t h u) -> p (t h) u", h=HPG, u=128)
        nc.gpsimd.memset(v4[:, :, 65:128], 0.0)
        nc.vector.memset(v4[:, :, 64:65], 1.0)

        # ---- QKV q/k: per-head packed [q(64) | k(64)] outputs ----
        qk_sb = []      # [128,T]: rows 0:64 q_h, 64:128 k_h
        qq_sb = []      # [128,T]: q_h duplicated to both row halves
        kk_sb = []      # [128,T]: k_h duplicated
        for h in range(HPG):
            qk_sb.append(sb.tile([128, T], bf16, tag=f"qk{h}", name=f"qk{h}"))
            qq_sb.append(sb.tile([128, T], bf16, tag=f"qq{h}", name=f"qq{h}"))
            kk_sb.append(sb.tile([128, T], bf16, tag=f"kk{h}", name=f"kk{h}"))

        def qk_unit(h, c):
            """q,k for head h, cols c:c+512 -> qk_sb[h]."""
            pq = fill_tile(f"pq{h}_{c}")
            for k in range(KT):
                nc.tensor.matmul(
                    pq[:, 0:512],
                    lhsT=wqkt_sb[:, k * (HPG * 128) + h * 128:
                                 k * (HPG * 128) + (h + 1) * 128],
                    rhs=xt3[:, k, c:c + 512],
                    start=(k == 0), stop=(k == KT - 1),
                )
            nc.vector.tensor_scalar_add(
                qk_sb[h][:, c:c + 512], pq[:, 0:512], bqk_sb[:, h:h + 1])

        def dup_unit(h, eng):
            """Duplicate q,k of head h across both row halves (4 DMAs)."""
            eng.dma_start(qq_sb[h][0:64, :], qk_sb[h][0:64, :])
            eng.dma_start(qq_sb[h][64:128, :], qk_sb[h][0:64, :])
            eng.dma_start(kk_sb[h][0:64, :], qk_sb[h][64:128, :])
            eng.dma_start(kk_sb[h][64:128, :], qk_sb[h][64:128, :])

        def v_unit(t):
            pv = fill_tile(f"pv{t}")
            for k in range(KT):
                nc.tensor.matmul(
                    pv[:, 0:CPG],
                    lhsT=xt3[:, k, t * 128:(t + 1) * 128],
                    rhs=wvt_sb[:, k * CPG:(k + 1) * CPG],
                    start=(k == 0), stop=(k == KT - 1),
                )
            nc.vector.tensor_add(
                v4[:, t * HPG:(t + 1) * HPG, 0:64],
                pv[:, 0:CPG].rearrange("p (h d) -> p h d", d=64),
                bv_sb[:, :].rearrange("p (h d) -> p h d", d=64),
            )

        # ---- attention blocks: (h, p), pass-major ----
        pt0 = sb.tile([128, T], bf16, tag="pt0", name="pt0")
        pt1 = sb.tile([128, T], bf16, tag="pt1", name="pt1")
        nc.gpsimd.memset(pt1[64:128, :], 0.0)
        p_slices = [(pt0, 0), (pt0, 64), (pt1, 0)]

        # pair schedule per pass: (A, B) tile indices; B occupies row group
        # 64 and sp cols 1024:2048. B is always the fuller tile.
        pairs_p = {
            0: [(1, 0), (3, 2), (5, 4), (7, 6)],
            1: [(8, 0), (9, 1), (10, 2), (11, 3),
                (12, 4), (13, 5), (14, 6), (15, 7)],
        }

        pending = []            # deferred small stages (run off PE path)

        def drain(n=99):
            for _ in range(min(n, len(pending))):
                pending.pop(0)()

        class Block:
            def __init__(self, h, p):
                self.h, self.p = h, p
                self.base = p * PW
                self.i_max = (self.base + PW) // 128
                self.ot = None
                self.done = set()      # tiles with O emitted
                self.started = set()   # psum banks of ot with first write
                self.normed = set()
                self.last = {0: min(self.i_max - 1, self.base // 128 + 3),
                             512: min(self.i_max - 1,
                                      (self.base + 512) // 128 + 3)}

            def get_ot(self):
                if self.ot is None:
                    self.ot = ps.tile([128, PW], f32, tag="ot", bufs=1,
                                      name=f"ot{self.h}_{self.p}")
                return self.ot

            def lo(self, i):
                return max(i * 128 - self.base, 0)

            def s_pair(self, j):
                a, b = pairs_p[self.p][j]
                sp = ps.tile([128, 2048], f32, tag="sp", bufs=1,
                             name=f"sp{self.h}_{self.p}_{j}")
                ex = sb.tile([128, 2048], bf16, tag="ex", bufs=4,
                             name=f"ex{self.h}_{self.p}_{j}")
                la, lb = self.lo(a), self.lo(b)
                for off, i, l, tp in ((0, a, la, 0), (1024, b, lb, 64)):
                    kv = kk_sb[self.h][tp:tp + 64, i * 128:(i + 1) * 128]
                    qv = qq_sb[self.h]
                    for b0 in (0, 512):
                        cs, ce = max(l, b0), b0 + 512
                        if cs >= ce:
                            continue
                        nc.tensor.matmul(
                            sp[:, off + cs:off + ce],
                            lhsT=kv,
                            rhs=qv[tp:tp + 64,
                                   self.base + cs:self.base + ce],
                            start=True, stop=True,
                            tile_position=(tp, 0),
                        )
                # exp: one instr if the dead gap [1024:1024+lb] is smaller
                # than the 352-cycle instruction overhead, else two
                if lb < 352:
                    nc.scalar.activation(ex[:, la:2048], sp[:, la:2048],
                                         AF.Exp, scale=SCALE)
                else:
                    nc.scalar.activation(ex[:, la:1024], sp[:, la:1024],
                                         AF.Exp, scale=SCALE)
                    nc.scalar.activation(ex[:, 1024 + lb:2048],
                                         sp[:, 1024 + lb:2048],
                                         AF.Exp, scale=SCALE)
                # causal masks of diagonal blocks (post-exp, bf16)
                for mi, (off, i) in enumerate(((0, a), (1024, b))):
                    r = i * 128 - self.base
                    if 0 <= r < PW:
                        eng = nc.gpsimd if (j + mi) % 2 else nc.vector
                        eng.tensor_mul(ex[:, off + r:off + r + 128],
                                       ex[:, off + r:off + r + 128],
                                       mask_sb[:, :])
                return ex

            def o_pair(self, j, ex):
                a, b = pairs_p[self.p][j]
                ot = self.get_ot()
                for off, i in ((0, a), (1024, b)):
                    l = self.lo(i)
                    for b0 in (0, 512):
                        cs, ce = max(l, b0), b0 + 512
                        if cs >= ce:
                            continue
                        self.done.add((i, b0))
                        stop = all(
                            (i2, b0) in self.done
                            for i2 in range(self.last[b0] + 1))
                        nc.tensor.matmul(
                            ot[:, cs:ce],
                            lhsT=vall[:, i * (HPG * 128) + self.h * 128:
                                      i * (HPG * 128) + (self.h + 1) * 128],
                            rhs=ex[:, off + cs:off + ce],
                            start=(b0 not in self.started), stop=stop,
                        )
                        self.started.add(b0)
                        if stop and b0 not in self.normed:
                            self.normed.add(b0)
                            self.norm(b0)

            def norm(self, b0):
                # rowsum (ot row 64) -> bf16 row; deferred: ones-matmul
                # broadcast, reciprocal, multiply into pdst
                ot = self.ot
                h, p, base = self.h, self.p, self.base
                rsb = sb.tile([1, 512], bf16, tag="rsb", bufs=4,
                              name=f"rsb{h}_{p}_{b0}")
                nc.vector.tensor_copy(rsb[:, :], ot[64:65, b0:b0 + 512])
                pdst, po = p_slices[h]

                def norm_b():
                    bs = fill_tile(f"bs{h}_{p}_{b0}")
                    nc.tensor.matmul(bs[0:64, 0:512], lhsT=ones_sb[:, 0:64],
                                     rhs=rsb[:, :], start=True, stop=True)
                    rb = sb.tile([64, 512], f32, tag="rb", bufs=2,
                                 name=f"rb{h}_{p}_{b0}")
                    nc.vector.reciprocal_approx_fast(rb[:, :], bs[0:64, 0:512])
                    nc.vector.tensor_mul(
                        pdst[po:po + 64, base + b0:base + b0 + 512],
                        ot[0:64, b0:b0 + 512], rb[:, :])
                pending.append(norm_b)

        def emit_proj_tile(t):
            ysb = sb.tile([128, C], bf16, tag=f"ysb{t % 2}", bufs=2,
                          name=f"ysb{t}")
            for n0, nn in ((0, 512), (512, 256)):
                yp = fill_tile(f"yp{t}_{n0}")
                nc.tensor.matmul(yp[:, 0:nn],
                                 lhsT=pt0[:, t * 128:(t + 1) * 128],
                                 rhs=wpt0_sb[:, n0:n0 + nn],
                                 start=True, stop=False)
                nc.tensor.matmul(yp[:, 0:nn],
                                 lhsT=pt1[:, t * 128:(t + 1) * 128],
                                 rhs=wpt1_sb[:, n0:n0 + nn],
                                 start=False, stop=True)
                nc.vector.tensor_copy(ysb[:, n0:n0 + nn], yp[:, 0:nn])
            nc.sync.dma_start(y_d[t * 128:(t + 1) * 128, :], ysb[:, 0:C])

        # ---- flat pipeline ----
        # Prerequisite-driven filler emission: before a block's first S-pair
        # its head's qk+dup must be emitted; before each O-pair, the v-units
        # it consumes. Leftover fillers dribble 1-2 per pair.
        qk_done = [False] * HPG
        v_done = [False] * NT

        def ensure_qk(h):
            if not qk_done[h]:
                qk_done[h] = True
                for c in range(4):
                    qk_unit(h, c * 512)
                dup_unit(h, nc.scalar if h == 0 else nc.sync)

        def ensure_v(tiles):
            for t in tiles:
                if not v_done[t]:
                    v_done[t] = True
                    v_unit(t)

        filler_q = [lambda h=h: ensure_qk(h) for h in (1, 2)] + \
                   [lambda t=t: ensure_v([t]) for t in range(NT)]

        # startup: head-0 q,k + first v tiles
        ensure_qk(0)
        ensure_v(range(0, 4))

        blocks = [Block(h, p) for p in range(2) for h in range(HPG)]
        proj_emitted = 0

        # stream: S(j) ... [O(j-1), fillers] ... S(j+1); across blocks the
        # S stream runs `depth` pair-units ahead of the O stream.
        stream = [(bi, j) for bi, blk in enumerate(blocks)
                  for j in range(len(pairs_p[blk.p]))]
        depth = 2
        ex_store = {}
        proj_ready = 0

        def emit_filler_batch(n):
            nonlocal proj_ready, proj_emitted
            for _ in range(n):
                if filler_q:
                    filler_q.pop(0)()
                elif proj_emitted < proj_ready:
                    drain()   # pt0/pt1 writers must be emitted first
                    emit_proj_tile(proj_emitted)
                    proj_emitted += 1
                else:
                    break

        def do_o(obi, oj):
            nonlocal proj_ready
            a, b = pairs_p[blocks[obi].p][oj]
            ensure_v([a, b])
            blocks[obi].o_pair(oj, ex_store.pop((obi, oj)))
            if (obi, oj) == (2, len(pairs_p[0]) - 1):
                proj_ready = 8           # all heads p0 normed (after drain)

        for idx, (bi, j) in enumerate(stream):
            drain()
            ensure_qk(blocks[bi].h)
            ex_store[(bi, j)] = blocks[bi].s_pair(j)
            if idx >= depth:
                do_o(*stream[idx - depth])
                emit_filler_batch(2)
            else:
                emit_filler_batch(1)
        for k in range(depth):
            do_o(*stream[len(stream) - depth + k])
            drain()
            emit_filler_batch(2)
        proj_ready = 16
        drain()
        while proj_emitted < 16:
            emit_proj_tile(proj_emitted)
            proj_emitted += 1
            drain()

    nc.finalize()
    return nc


def _get_module():
    if "nc" not in _CACHE:
        _CACHE["nc"] = _build_module()
    return _CACHE["nc"]


def make_in_maps(x, w_attn, b_attn, w_proj):
    """Host-side sharding: per-core input dicts (8 cores)."""
    import ml_dtypes
    bf16 = ml_dtypes.bfloat16
    x = np.asarray(x, dtype=np.float32)
    w_attn = np.asarray(w_attn, dtype=np.float32)
    b_attn = np.asarray(b_attn, dtype=np.float32)
    w_proj = np.asarray(w_proj, dtype=np.float32)

    xts = [np.ascontiguousarray(x[b].T).astype(bf16) for b in range(B)]
    mask = np.triu(np.ones((128, 128), np.float32)).astype(bf16)

    in_maps = []
    for c in range(8):
        b = c // G
        hg = c % G
        sl = slice(CPG * hg, CPG * (hg + 1))
        wq = w_attn[0:C][sl]          # [192, 768]
        wk = w_attn[C:2 * C][sl]
        wv = w_attn[2 * C:3 * C][sl]
        bq = b_attn[0:C][sl]
        bk = b_attn[C:2 * C][sl]
        bv = b_attn[2 * C:3 * C][sl]
        # per head h: wqkt cols h*128..+128 = [wq_h (64) | wk_h (64)]
        wqkt = np.zeros((C, HPG * 128), np.float32)
        bqk = np.zeros((128, HPG), np.float32)
        for h in range(HPG):
            wqkt[:, h * 128:h * 128 + 64] = wq[h * 64:(h + 1) * 64].T
            wqkt[:, h * 128 + 64:(h + 1) * 128] = wk[h * 64:(h + 1) * 64].T
            bqk[0:64, h] = bq[h * 64:(h + 1) * 64]
            bqk[64:128, h] = bk[h * 64:(h + 1) * 64]
        wvt = np.ascontiguousarray(wv.T).astype(bf16)             # [768, 192]
        bvb = np.ascontiguousarray(
            np.broadcast_to(bv, (128, CPG))).astype(np.float32)   # [128, 192]
        wpt = np.ascontiguousarray(w_proj[:, sl].T).astype(bf16)  # [192, 768]
        in_maps.append({
            "xt": xts[b],
            "wqkt": wqkt.astype(bf16),
            "wvt": wvt,
            "bqk": bqk,
            "bv": bvb,
            "wpt": wpt,
            "mask": mask,
        })
    return in_maps


def gather(results, b_proj):
    """Sum the 4 head-group partials per batch, add bias."""
    b_proj = np.asarray(b_proj, dtype=np.float32)
    y = np.zeros((B, T, C), np.float32)
    for c in range(8):
        y[c // G] += np.asarray(results[c]["y"], dtype=np.float32)
    y += b_proj
    return y


def run(x, w_attn, b_attn, w_proj, b_proj, trace=False, **kw):
    from concourse.bass_utils import run_bass_kernel_spmd
    nc = _get_module()
    in_maps = make_in_maps(x, w_attn, b_attn, w_proj)
    res = run_bass_kernel_spmd(nc, in_maps, list(range(8)), trace=trace, **kw)
    return gather(res.results, b_proj), res


def kernel(x, w_attn, b_attn, w_proj, b_proj):
    y, _ = run(x, w_attn, b_attn, w_proj, b_proj)
    return y
